# revision 41
# baseline (speedup 1.0000x reference)
"""Single-head causal attention (B=8, T=2048, C=384, H=64) on 8 NeuronCores.

Data-parallel over batch: core b computes attention for batch element b.
Per-core pipeline (all matmuls bf16, fp32 PSUM accumulate):
  - host pre-transposes x -> xT [C, T] bf16; W_qk = [Wq|Wk] fused [C, 128]
  - fused qk projection: one M=128 matmul pass (psum rows 0:64 = qT,
    64:128 = kT); DVE casts q -> qq[64, T], ACT casts k (partition shift
    64->0) -> kk[64, T]
  - v = x @ Wv (48 matmuls, N=64) -> v_sb [128, 16, 66] bf16 + ones col
  - S(j) = kk_j.T @ qq  [128, W] psum chunks of 1024
  - exp: split between ACT (true Exp, scale=1/sqrt(C), bf16 out) and DVE
    (Schraudolph int16 bit-trick: i16 = S*A16 + B16, bitcast bf16;
    diagonal chunks get a per-element bias-mask via scalar_tensor_tensor,
    masked lanes saturate to 0x8000 = -0.0). Pool multiplies the diagonal
    block by a 0/1 mask for ACT-exp'd diagonal chunks.
  - PV: out_i = sum_j pt_j[:, block i].T @ [v_j | 1]  (PSUM accumulate;
    ones column gives the softmax denominator in col 64)
  - normalize 4 row-blocks at a time: DVE reciprocal [128,4] + one
    broadcast tensor_tensor multiply; DMA f32 out
"""

import math
import os

import numpy as np
import ml_dtypes

import concourse.bass as bass
import concourse.tile as tile
from concourse import bacc, mybir
from concourse.bass import ds, ts
from concourse.bass_utils import run_bass_kernel_spmd

F32 = mybir.dt.float32
BF16 = mybir.dt.bfloat16
I16 = mybir.dt.int16

B, T, C, H = 8, 2048, 384, 64
P = 128
NT = T // P          # 16 t-tiles
NCC = C // P         # 3 contraction chunks
SCALE = 1.0 / math.sqrt(float(C))

# Schraudolph-style exp2 bit trick constants (bf16 = int16 bit pattern):
# i16 = round(S * A16 + B16) -> bitcast bf16 ~= exp(S * SCALE)
# c = -6.0 calibrated to remove the mean bias vs true exp.
A16 = 128.0 * math.log2(math.e) * SCALE
B16 = 127.0 * 128.0 - 6.0
NEG_BIG = -1.0e9   # masked lanes saturate i16 -> -32768 = bf16 -0.0

LAST_RESULT = None
_PROGRAM = None

Mult = mybir.AluOpType.mult
Add = mybir.AluOpType.add


def _emit(tc: tile.TileContext, xT_d, wqk_d, wv_d, bm_d, mask_d, out_d, ctx):
    nc = tc.nc
    Exp = mybir.ActivationFunctionType.Exp

    const = ctx.enter_context(tc.tile_pool(name="const", bufs=1))
    big = ctx.enter_context(tc.tile_pool(name="big", bufs=1))
    outp = ctx.enter_context(tc.tile_pool(name="outp", bufs=2))
    ps = ctx.enter_context(tc.tile_pool(name="ps", bufs=1, space="PSUM"))

    # ---- input DMAs -------------------------------------------------------
    # two hardware queues: sync (SP) carries xT c0/c1, scalar (ACT) carries
    # the weights + xT c2; pieces ordered by first use.
    xT = [
        big.tile([P, T], BF16, tag=f"xT{c}", name=f"xT{c}")
        for c in range(NCC)
    ]
    wqk_sb = const.tile([P, NCC, P], BF16, tag="wqk")
    wv_sb = const.tile([P, NCC, H], BF16, tag="wv")
    bm_sb = const.tile([P, P], F32, tag="bm")
    mask_sb = const.tile([P, P], BF16, tag="mask")

    # warm tile first: its memset must not queue behind DMA dispatches
    warm = big.tile([P, 256], BF16, tag="warm")
    nc.vector.memset(warm[:], 0.25)

    # two DMA queues (sync / scalar), pieces ordered by first use: the
    # projection consumes 512-column slices in order, so land each slice of
    # all three c-chunks before any later slice. mask/bm are tiny and are
    # needed by the first diagonal exp op, so they go early on scalar.
    nc.scalar.dma_start(wqk_sb[:], wqk_d.rearrange("(c p) m -> p c m", p=P))
    nc.scalar.dma_start(xT[2][:, ts(0, 512)], xT_d[2 * P : 3 * P, ts(0, 512)])
    nc.sync.dma_start(xT[0][:, ts(0, 512)], xT_d[0:P, ts(0, 512)])
    nc.sync.dma_start(xT[1][:, ts(0, 512)], xT_d[P : 2 * P, ts(0, 512)])
    nc.sync.dma_start(bm_sb[:], bm_d[:])
    nc.scalar.dma_start(
        xT[2][:, ds(512, 1536)], xT_d[2 * P : 3 * P, ds(512, 1536)]
    )
    nc.sync.dma_start(xT[0][:, ts(1, 512)], xT_d[0:P, ts(1, 512)])
    nc.sync.dma_start(xT[1][:, ts(1, 512)], xT_d[P : 2 * P, ts(1, 512)])
    nc.gpsimd.dma_start(xT[0][:, ds(1024, 1024)], xT_d[0:P, ds(1024, 1024)])
    nc.gpsimd.dma_start(
        xT[1][:, ds(1024, 1024)], xT_d[P : 2 * P, ds(1024, 1024)]
    )
    nc.sync.dma_start(wv_sb[:], wv_d.rearrange("(c p) h -> p c h", p=P))
    nc.sync.dma_start(mask_sb[:], mask_d[:])

    # ---- fused q/k projection --------------------------------------------
    # psum rows 0:64 = qT rows, 64:128 = kT rows (W_qk = [Wq | Wk])
    qq = big.tile([H, T], BF16, tag="qq")
    kk = big.tile([H, T], BF16, tag="kk")

    def emit_proj(t4):
        pq = ps.tile([P, 512], F32, tag="st", bufs=3, name=f"pq{t4}")
        for c in range(NCC):
            nc.tensor.matmul(
                pq[:], wqk_sb[:, c, :], xT[c][:, ts(t4, 512)],
                start=(c == 0), stop=(c == NCC - 1),
            )
        nc.vector.tensor_copy(qq[:, ts(t4, 512)], pq[0:H, :])
        nc.scalar.copy(kk[:, ts(t4, 512)], pq[H:P, :])

    # ---- exp engine balancer ---------------------------------------------
    est = {"act": 3.4, "dve": 5.6}   # fixed work preload (us)

    def emit_exp(pt_tile, st_tile, c0, w, diag):
        cost_a = (w * 0.833 + 200.0) / 1000.0
        cost_d = (w * 1.04 + 200.0 + (250.0 if diag else 0.0)) / 1000.0
        if est["act"] + cost_a <= est["dve"] + cost_d:
            est["act"] += cost_a
            nc.scalar.activation(
                pt_tile[:, ds(c0, w)], st_tile[:, 0:w], Exp, scale=SCALE
            )
            if diag:
                nc.gpsimd.tensor_tensor(
                    pt_tile[:, 0:P], pt_tile[:, 0:P], mask_sb[:], Mult
                )
        else:
            est["dve"] += cost_d
            if diag:
                # per-element bias-mask on the 128-wide causal diagonal
                nc.vector.scalar_tensor_tensor(
                    pt_tile[:, 0:P].bitcast(I16), st_tile[:, 0:P],
                    A16, bm_sb[:], Mult, Add,
                )
                if w > P:
                    nc.vector.tensor_scalar(
                        pt_tile[:, ds(c0 + P, w - P)].bitcast(I16),
                        st_tile[:, ds(P, w - P)], A16, B16, Mult, Add,
                    )
            else:
                nc.vector.tensor_scalar(
                    pt_tile[:, ds(c0, w)].bitcast(I16), st_tile[:, 0:w],
                    A16, B16, Mult, Add,
                )

    # ---- score pass S(j) --------------------------------------------------
    pt_tiles = {}

    def round_up_32(size):
        for v in (32, 64, 128):
            if v >= size:
                return v

    def matmul_noload(out, lhsT, rhs):
        """matmul marked non-self-loading: reuses the PE array weights left
        by the immediately preceding matmul (which had the same lhsT)."""
        te = nc.tensor
        kd = {0}
        ifmap_ap = te.lower_ap(rhs.opt(kd), opt=False)
        weights_ap = te.lower_ap(lhsT.opt(kd), opt=False,
                                 for_matmul_weights=True)
        return te.add_instruction(
            mybir.InstMatmult(
                name=nc.get_next_instruction_name(),
                replication_resolution=0,
                replication_shift_amnt=0,
                replication_num_rows=0,
                start_tensor_calc=True,
                stop_tensor_calc=True,
                ins=[ifmap_ap, weights_ap],
                outs=[te.lower_ap(out)],
                perf_mode=None,
                is_transpose=None,
                ifmap_quant_offset=None,
                weights_quant_offset=None,
                bass_skip_group_check=False,
                tile_position=(lhsT.base_partition(), out.base_partition()),
                tile_size=(round_up_32(rhs.partition_size()),
                           round_up_32(out.partition_size())),
                ldweights=False,
            )
        )

    def emit_S_chunk(j, off):
        t0 = P * j
        W = T - t0
        if off == 0:
            pt = big.tile([P, W], BF16, tag=f"pt{j}", name=f"pt{j}")
            pt_tiles[j] = pt
        pt = pt_tiles[j]
        w = min(1024, W - off)
        st = ps.tile([P, 1024], F32, tag="st", bufs=3, name=f"st{j}_{off}")
        for o2 in range(0, w, 512):
            n2 = min(512, w - o2)
            if o2 == 0:
                nc.tensor.matmul(
                    st[:, ds(o2, n2)], kk[:, ds(t0, P)],
                    qq[:, ds(t0 + off + o2, n2)], start=True, stop=True,
                )
            else:
                matmul_noload(
                    st[:, ds(o2, n2)], kk[:, ds(t0, P)],
                    qq[:, ds(t0 + off + o2, n2)],
                )
        emit_exp(pt, st, off, w, diag=(off == 0))

    def n_chunks(j):
        return (T - P * j + 1023) // 1024

    # ---- v projection (+ ones column) ------------------------------------
    v_sb = big.tile([P, NT, 66], BF16, tag="v")

    def emit_V():
        pv = ps.tile([P, 1024], F32, tag="st", bufs=3, name="pv")
        for j in range(NT):
            for c in range(NCC):
                nc.tensor.matmul(
                    pv[:, ts(j, H)], xT[c][:, ds(P * j, P)], wv_sb[:, c, :],
                    start=(c == 0), stop=(c == NCC - 1),
                )
        nc.scalar.copy(
            v_sb[:, :, 0:H], pv[:].rearrange("p (j h) -> p j h", h=H)
        )
        nc.gpsimd.memset(v_sb[:, :, H:65], 1.0)

    # ---- output pass PV(i) ------------------------------------------------
    out_v = out_d.rearrange("(g k p) h -> g p k h", p=P, k=4)
    oa4 = [None]
    ob4 = [None]

    def emit_PV(i, js):
        if not js:
            return
        if i % 4 == 0 and js[0] == 0:
            oa4[0] = ps.tile([P, 4, 72], F32, tag="oa", bufs=2,
                             name=f"oa{i // 4}")
            ob4[0] = outp.tile([P, 4, H], F32, tag="ob", bufs=2,
                               name=f"ob{i // 4}")
        oa = oa4[0]
        for j in js:
            nc.tensor.matmul(
                oa[:, i % 4, 0:65], pt_tiles[j][:, ds(P * (i - j), P)],
                v_sb[:, j, 0:65], start=(j == 0), stop=(j == i),
            )
        if i % 4 == 3 and js[-1] == i:
            g = i // 4
            rec = outp.tile([P, 4], F32, tag="rec", bufs=2, name=f"rec{g}")
            nc.vector.reciprocal(rec[:], oa[:, :, 64])
            nc.vector.tensor_tensor(
                ob4[0][:], oa[:, :, 0:H],
                rec[:].unsqueeze(2).broadcast_to([P, 4, H]), Mult,
            )
            nc.sync.dma_start(out_v[g], ob4[0][:])

    # ---- main loop --------------------------------------------------------
    # dependency-free warmup/filler matmuls on the memset tile: keep the PE
    # busy during the input-DMA window (DVFS clock ramp) and plug the
    # cast-latency bubbles between projection groups. They share the "st"
    # slots, so only place them where the 3-back tile's consumer is a quick
    # cast (not exp).
    wcount = [0]

    def filler(n):
        for _ in range(n):
            r = wcount[0]
            wcount[0] += 1
            pw = ps.tile([P, 256], F32, tag="st", bufs=3, name=f"pw{r}")
            nc.tensor.matmul(pw[:], warm[:, 0:P], warm[:], start=True,
                             stop=True)

    filler(8)
    for t4 in range(4):
        emit_proj(t4)
        filler(2)
    emit_S_chunk(0, 0)
    emit_S_chunk(0, 1024)
    emit_S_chunk(1, 0)
    emit_S_chunk(1, 1024)
    emit_V()
    # steady state: interleave PV(i-2) matmuls between S(i)'s chunks so the
    # PE has fill work while exp drains the score psum buffers
    for i in range(2, NT):
        pv = i - 2
        js = list(range(pv + 1))
        half = (len(js) + 1) // 2
        emit_S_chunk(i, 0)
        emit_PV(pv, js[:half])
        for off in range(1024, T - P * i, 1024):
            emit_S_chunk(i, off)
        emit_PV(pv, js[half:])
    emit_PV(NT - 2, list(range(NT - 1)))
    emit_PV(NT - 1, list(range(NT)))


def _build_program():
    nc = bacc.Bacc("TRN2", target_bir_lowering=False, debug=False, num_devices=B)
    xT_d = nc.dram_tensor("xT", [C, T], BF16, kind="ExternalInput").ap()
    wqk_d = nc.dram_tensor("wqk", [C, P], BF16, kind="ExternalInput").ap()
    wv_d = nc.dram_tensor("wv", [C, H], BF16, kind="ExternalInput").ap()
    bm_d = nc.dram_tensor("bm", [P, P], F32, kind="ExternalInput").ap()
    mask_d = nc.dram_tensor("mask", [P, P], BF16, kind="ExternalInput").ap()
    out_d = nc.dram_tensor("out", [T, H], F32, kind="ExternalOutput").ap()
    from contextlib import ExitStack

    with tile.TileContext(nc) as tc:
        with ExitStack() as ctx:
            _emit(tc, xT_d, wqk_d, wv_d, bm_d, mask_d, out_d, ctx)
    nc.compile()
    return nc


def kernel(x, Wq, Wk, Wv):
    global LAST_RESULT, _PROGRAM
    assert x.shape == (B, T, C), x.shape
    if _PROGRAM is None:
        _PROGRAM = _build_program()
    nc = _PROGRAM

    bf = ml_dtypes.bfloat16
    xT = np.ascontiguousarray(np.transpose(x, (0, 2, 1))).astype(bf)
    wqk = np.concatenate([Wq, Wk], axis=1).astype(bf)
    wv = np.ascontiguousarray(Wv).astype(bf)

    # bias-mask for DVE diagonal blocks: keep s <= t, else drive the int16
    # trick into saturation (-32768 = bf16 -0.0)
    s_idx = np.arange(P)[:, None]
    t_idx = np.arange(P)[None, :]
    bm = np.where(s_idx <= t_idx, B16, NEG_BIG).astype(np.float32)
    # 0/1 multiplicative mask for ACT-exp'd diagonal blocks
    mask = np.triu(np.ones((P, P), dtype=np.float32)).astype(bf)

    in_maps = [
        {"xT": xT[b], "wqk": wqk, "wv": wv, "bm": bm, "mask": mask}
        for b in range(B)
    ]
    trace = bool(int(os.environ.get("KERNEL_TRACE", "0")))
    kw = {}
    td = os.environ.get("KERNEL_TRACE_DIR")
    if td:
        kw["tmpdir"] = td
    LAST_RESULT = run_bass_kernel_spmd(
        nc, in_maps, list(range(B)), trace=trace, **kw
    )
    out = np.stack([LAST_RESULT.results[b]["out"] for b in range(B)], axis=0)
    return out.astype(np.float32)
